# revision 16
# baseline (speedup 1.0000x reference)
"""Trainium2 Bass kernel for nn_ASD: LSTMCell over T=512 steps + linear readout.

Sharding: the 4096 gate columns (= 1024 hidden units x 4 gates) are sharded
8 ways; core p owns hidden units [128p, 128(p+1)) and computes their i/f/o/g
gates, cell state and hidden state. Each step every core broadcasts its h
shard [128, B] (bf16) into slot p of every peer's gathered-h buffer with a
single remote_dma_broadcast (SWDGE descriptors, all 16 SDMA engines, 2 per
destination), so the per-step exchange costs ~1-2us instead of the ~10us+
of a collective_compute AllGather. Cross-core ordering is enforced with two
raw semaphores (recv: all 8 shards of step t landed; src: my previous
broadcast drained so its source buffer can be rewritten) attached as
post-compile wait conditions, because the Tile scheduler cannot see remote
writes.

Everything on-chip runs "transposed": hidden/gate units on SBUF partitions,
batch on the free dim. Phase A precomputes P = x @ W_ih.T + b for this
core's gate columns at full PE utilization; phase B consumes P per step via
a PSUM-injecting identity matmul, adds the recurrent term with 32 accumulating
128x128 matmuls over the gathered h (full batch N=32, one weight pass per
step), applies the LSTM cell, broadcasts the new h shard, and accumulates the
tiny readout matmul.

Compute dtype bf16 (PSUM accumulation fp32, cell state fp32).
"""
import os
import sys
import numpy as np
import ml_dtypes

for _p in ("/opt/trn_rl_repo", "/root/.axon_site/_ro/trn_rl_repo"):
    if _p not in sys.path and os.path.isdir(_p):
        sys.path.append(_p)

import concourse.bass as bass
import concourse.bacc as bacc
import concourse.mybir as mybir
import concourse.tile as tile
from concourse.instruction_name_ordered_set import InstructionNameOrderedSet
from concourse.bass_utils import run_bass_kernel_spmd

BF16 = ml_dtypes.bfloat16
N_CORES = 8
B = 32          # batch
F = 2048        # input features
H = 1024        # hidden
T_FULL = 512    # sequence length
KF = F // 128   # 16 feature chunks
KH = H // 128   # 8 hidden chunks (= slots, one per core)
GRP = 16        # timesteps per projection group
FP32 = mybir.dt.float32
DBF16 = mybir.dt.bfloat16
AF = mybir.ActivationFunctionType


def build_nc(T=T_FULL, comm="rdma", phase_b=True, reps=1):
    """comm: 'rdma' (remote_dma_broadcast h-exchange), 'cc' (AllGather
    collective), 'none' (timing ablation only - wrong results)."""
    n_grp = T // GRP
    nc = bacc.Bacc("TRN2", target_bir_lowering=False, debug=False,
                   num_devices=N_CORES)

    xt = nc.declare_dram_parameter("xt", [F, T * B], DBF16, isOutput=False)
    w_ih = nc.declare_dram_parameter("w_ih", [F, 512], DBF16, isOutput=False)
    w_hh = nc.declare_dram_parameter("w_hh", [H, 512], DBF16, isOutput=False)
    bias = nc.declare_dram_parameter("bias", [4, 128], FP32, isOutput=False)
    w_fc = nc.declare_dram_parameter("w_fc", [128, 2], DBF16, isOutput=False)
    ident = nc.declare_dram_parameter("ident", [128, 128], DBF16, isOutput=False)
    out = nc.declare_dram_parameter("out", [2, T * B], FP32, isOutput=True)

    # raw cross-core semaphores (invisible to Tile; waits patched post-compile)
    recv_sem = nc.alloc_semaphore("rdma_recv_sem")
    src_sem = nc.alloc_semaphore("rdma_src_sem")
    pending_waits = []  # (BassInstruction, sem, value)

    with tile.TileContext(nc) as tc:
        with (
            tc.tile_pool(name="wpool", bufs=1) as wpool,
            tc.tile_pool(name="xpool", bufs=32) as xpool,
            tc.tile_pool(name="pa_ps", bufs=2, space="PSUM") as pa_ps,
            tc.tile_pool(name="pa_out", bufs=3) as pa_out,
            tc.tile_pool(name="ptpool", bufs=2) as ptpool,
            tc.tile_pool(name="g_ps", bufs=2, space="PSUM") as g_ps,
            tc.tile_pool(name="act", bufs=2) as actp,
            tc.tile_pool(name="cell", bufs=2) as cellp,
            tc.tile_pool(name="state", bufs=1) as statep,
            tc.tile_pool(name="ro_ps", bufs=2, space="PSUM") as ro_ps,
            tc.tile_pool(name="ro_sb", bufs=2) as ro_sbp,
            tc.tile_pool(name="dram", bufs=2, space="DRAM") as dramp,
        ):
            if comm == "rdma":
                # NRT resets semaphores to 0 on every nrt_execute (absolute
                # Tile sem waits rely on this too), so recv/src start at 0.
                # column offset of this core's slot in peers' gathered-h
                bcast_off = nc.gpsimd.partition_id() * B

            # ---- load weights/constants ----
            w_ih_sb = wpool.tile([128, KF * 512], DBF16, tag="w_ih")
            for k in range(KF):
                nc.sync.dma_start(w_ih_sb[:, k * 512:(k + 1) * 512],
                                  w_ih[k * 128:(k + 1) * 128, :])
            w_hh_sb = wpool.tile([128, KH * 512], DBF16, tag="w_hh")
            for k in range(KH):
                nc.sync.dma_start(w_hh_sb[:, k * 512:(k + 1) * 512],
                                  w_hh[k * 128:(k + 1) * 128, :])
            ident_sb = wpool.tile([128, 128], DBF16, tag="ident")
            nc.sync.dma_start(ident_sb[:], ident[:])
            wfc_sb = wpool.tile([128, 2], DBF16, tag="wfc")
            nc.sync.dma_start(wfc_sb[:], w_fc[:])
            bias_sb = wpool.tile([128, 4], FP32, tag="bias")
            for m in range(4):
                nc.sync.dma_start(bias_sb[:, m:m + 1], bias[m, :][:, None])

            # gathered h, double-buffered: slot s = hidden units of core s
            hT = [statep.tile([128, KH * B], DBF16, tag=f"hT{pb}",
                              name=f"hT{pb}") for pb in range(2)]
            # local h shard (broadcast source / readout rhs), double-buffered
            hsrc = [statep.tile([128, B], DBF16, tag=f"hsrc{pb}",
                                name=f"hsrc{pb}") for pb in range(2)]

            n_bcast = 0  # broadcasts emitted so far (this core)
            for rep in range(reps):
                pt = dramp.tile([n_grp, 128, 4, GRP, B], DBF16,
                                tag="pt", name="pt")

                # ---- phase A (interleaved): P = x @ W_ih.T + b ----
                # Emitted in slices from inside the phase-B loop so the
                # projection matmuls fill the PE-idle window of each step's
                # AllGather (and keep HAM unthrottled). Group g is produced
                # two groups ahead of its consumption at step 16*g.
                xt_tiles_by_grp = {}

                def emit_xloads(grp, q):
                    tiles = xt_tiles_by_grp.setdefault(grp, [])
                    for k in range(4 * q, 4 * q + 4):
                        xtile = xpool.tile([128, 512], DBF16, tag="xt",
                                           name="xtile")
                        nc.sync.dma_start(
                            xtile[:],
                            xt[k * 128:(k + 1) * 128,
                               grp * GRP * B:(grp + 1) * GRP * B])
                        tiles.append(xtile)

                def emit_mtile(grp, m):
                    tiles = xt_tiles_by_grp[grp]
                    ps = pa_ps.tile([128, 512], FP32, tag="pa")
                    for k in range(KF):
                        nc.tensor.matmul(
                            ps[:],
                            w_ih_sb[:, k * 512 + m * 128:k * 512 + (m + 1) * 128],
                            tiles[k][:],
                            start=(k == 0), stop=(k == KF - 1))
                    ob = pa_out.tile([128, 512], DBF16, tag="pa_out")
                    nc.scalar.activation(ob[:], ps[:], AF.Identity,
                                         bias=bias_sb[:, m:m + 1])
                    nc.sync.dma_start(pt[grp, :, m, :, :], ob[:])
                    if m == 3:
                        del xt_tiles_by_grp[grp]

                def emit_group(grp):
                    for q in range(4):
                        emit_xloads(grp, q)
                    for m in range(4):
                        emit_mtile(grp, m)

                def emit_phase_a_slice(t):
                    grp = t // GRP + 2
                    if grp >= n_grp:
                        return
                    f = t % GRP
                    if f < 4:
                        emit_xloads(grp, f)
                    elif f in (4, 7, 10, 13):
                        emit_mtile(grp, (f - 4) // 3)

                for grp in range(min(2, n_grp)):
                    emit_group(grp)

                # ---- phase B: recurrence ----
                if not phase_b:
                    for grp in range(2, n_grp):
                        emit_group(grp)
                    for i in range(T * B // 512):
                        dummy = ro_sbp.tile([2, 512], FP32, tag="ro_out",
                                            name="dummy")
                        nc.gpsimd.memset(dummy[:], 0.0)
                        nc.sync.dma_start(out[:, i * 512:(i + 1) * 512],
                                          dummy[:])
                    continue

                if rep == 0:
                    # h_{-1} = 0 (reps>0 reuse stale h: timing-only builds).
                    # hT[1] is remote-written in rdma mode; memset it only
                    # when no exchange runs (avoids racing peers' broadcasts).
                    nc.gpsimd.memset(hT[0][:], 0.0)
                    if comm == "none":
                        nc.gpsimd.memset(hT[1][:], 0.0)
                c_prev = cellp.tile([128, B], FP32, tag="c", name="c")
                nc.gpsimd.memset(c_prev[:], 0.0)

                pt_grp = None
                ro_tile = None
                prev_last_mm = None
                for t in range(T):
                    par = t % 2
                    if t % GRP == 0:
                        pt_grp = ptpool.tile([128, 4 * GRP * B], DBF16,
                                             tag="ptg", name="ptg")
                        nc.sync.dma_start(pt_grp[:], pt[t // GRP])
                        ro_tile = ro_ps.tile([2, GRP * B], FP32, tag="ro",
                                             name="ro")
                    pt_view = pt_grp[:].rearrange("p (m t b) -> p m t b",
                                                  m=4, t=GRP, b=B)
                    # gates PSUM: partitions = unit-in-shard, free = (m, b)
                    ps = g_ps.tile([128, 4 * B], FP32, tag="g", name="g")
                    ident_mm = nc.tensor.matmul(
                        ps[:].rearrange("p (m b) -> p m b", m=4),
                        ident_sb[:], pt_view[:, :, t % GRP, :],
                        start=True, stop=False)
                    recv_nop = None
                    if comm == "rdma" and n_bcast > 0:
                        # gate the PE queue: all 8 shards of step t-1 landed.
                        # The sem wait is attached post-compile (invisible to
                        # the scheduler), so pin the nop's queue position with
                        # nosync deps: last K-loop matmul of step t-1 -> nop
                        # -> first K-loop matmul of step t.
                        recv_nop = nc.tensor.nop(nofuse=True, hint="recv_wait")
                        if prev_last_mm is not None:
                            recv_nop.ins.add_nosync_dependencies_from(
                                InstructionNameOrderedSet(
                                    [prev_last_mm.ins.name]))
                        pending_waits.append((recv_nop, recv_sem,
                                              16 * n_bcast))
                    # recurrent term over gathered h (slot s = core s)
                    for m in range(4):
                        for s in range(KH):
                            mm = nc.tensor.matmul(
                                ps[:, m * B:(m + 1) * B],
                                w_hh_sb[:, s * 512 + m * 128:
                                        s * 512 + (m + 1) * 128],
                                hT[par][:, s * B:(s + 1) * B],
                                start=False,
                                stop=(m == 3 and s == KH - 1))
                            if m == 0 and s == 0 and recv_nop is not None:
                                mm.ins.add_nosync_dependencies_from(
                                    InstructionNameOrderedSet(
                                        [recv_nop.ins.name]))
                    prev_last_mm = mm

                    # activations: i,f,o sigmoid (cols 0..3B), g tanh
                    sig = actp.tile([128, 3 * B], FP32, tag="sig", name="sig")
                    nc.scalar.activation(sig[:], ps[:, :3 * B], AF.Sigmoid)
                    tg = actp.tile([128, B], FP32, tag="tg", name="tg")
                    nc.scalar.activation(tg[:], ps[:, 3 * B:], AF.Tanh)
                    # cell update: c = f*c + i*g ; h = o*tanh(c)
                    fc = cellp.tile([128, B], FP32, tag="fc", name="fc")
                    nc.vector.tensor_mul(fc[:], sig[:, B:2 * B], c_prev[:])
                    ig = cellp.tile([128, B], FP32, tag="ig", name="ig")
                    nc.vector.tensor_mul(ig[:], sig[:, :B], tg[:])
                    c_new = cellp.tile([128, B], FP32, tag="c", name="c")
                    cadd = nc.vector.tensor_add(c_new[:], fc[:], ig[:])
                    tc_t = cellp.tile([128, B], FP32, tag="tc", name="tc")
                    nc.scalar.activation(tc_t[:], c_new[:], AF.Tanh)
                    src_nop = None
                    if comm == "rdma" and n_bcast >= 2:
                        # gate the DVE queue just before the hsrc[par] write:
                        # broadcast n_bcast-2 (the last reader of hsrc[par])
                        # has drained its source reads. Pinned between this
                        # step's c-add and the h-mul via nosync deps.
                        src_nop = nc.vector.nop(nofuse=True, hint="src_wait")
                        src_nop.ins.add_nosync_dependencies_from(
                            InstructionNameOrderedSet([cadd.ins.name]))
                        pending_waits.append((src_nop, src_sem,
                                              16 * (n_bcast - 1)))
                    hmul = nc.vector.tensor_mul(hsrc[par][:],
                                                sig[:, 2 * B:3 * B], tc_t[:])
                    if src_nop is not None:
                        hmul.ins.add_nosync_dependencies_from(
                            InstructionNameOrderedSet([src_nop.ins.name]))
                    c_prev = c_new

                    # readout (partial over this core's 128 hidden units)
                    nc.tensor.matmul(
                        ro_tile[:, (t % GRP) * B:(t % GRP + 1) * B],
                        wfc_sb[:], hsrc[par][:], start=True, stop=True)

                    # exchange h shards (skip after last step)
                    if t < T - 1:
                        if comm == "rdma":
                            nc.gpsimd.remote_dma_broadcast(
                                hT[1 - par][:, bass.ds(bcast_off, B)],
                                hsrc[par][:],
                                remote_sem=recv_sem, local_sem=src_sem,
                                rdests=[(0, k) for k in range(N_CORES)])
                            nc.gpsimd.trigger_dma(count=None)
                            n_bcast += 1
                        elif comm == "cc":
                            bin_t = dramp.tile([128, B], DBF16, tag="cc_in",
                                               name="cc_in")
                            bout_t = dramp.tile([N_CORES * 128, B], DBF16,
                                                tag="cc_out", name="cc_out",
                                                addr_space="Shared")
                            nc.sync.dma_start(bin_t[:], hsrc[par][:])
                            nc.gpsimd.collective_compute(
                                "AllGather", mybir.AluOpType.bypass,
                                ins=[bin_t[:]], outs=[bout_t[:]],
                                replica_groups=[list(range(N_CORES))])
                            nc.sync.dma_start(
                                hT[1 - par][:].rearrange("p (k b) -> p k b",
                                                         k=KH),
                                bout_t[:].rearrange("(k p) b -> p k b",
                                                    p=128))

                    # phase-A slice for group t//GRP + 2 fills the PE-idle
                    # window of this step's exchange
                    emit_phase_a_slice(t)

                    if t % GRP == GRP - 1:
                        ro_out = ro_sbp.tile([2, GRP * B], FP32, tag="ro_out",
                                             name="ro_out")
                        nc.scalar.activation(ro_out[:], ro_tile[:], AF.Copy)
                        nc.sync.dma_start(
                            out[:, (t - GRP + 1) * B:(t + 1) * B], ro_out[:])

    nc.compile()
    if not os.environ.get("KERNEL_SKIP_RDMA_WAITS"):
        for inst, sem, val in pending_waits:
            inst.wait_op(sem, val, "sem-ge")
    return nc


def make_in_maps(x, W_ih, W_hh, b_ih, b_hh, W_fc, T=T_FULL):
    """Per-core input shards. Gate-chunk order m = [i, f, o, g]; gathered-h
    slot order is absolute (slot s = hidden units of core s)."""
    xt = np.ascontiguousarray(
        x[:, :T, :].transpose(2, 1, 0).reshape(F, T * B)).astype(BF16)
    bsum = (b_ih + b_hh).astype(np.float32)
    eye = np.eye(128, dtype=np.float32).astype(BF16)
    in_maps = []
    for p in range(N_CORES):
        rows = np.concatenate([
            np.arange(p * 128, (p + 1) * 128),            # i
            np.arange(H + p * 128, H + (p + 1) * 128),    # f
            np.arange(3 * H + p * 128, 3 * H + (p + 1) * 128),  # o
            np.arange(2 * H + p * 128, 2 * H + (p + 1) * 128),  # g
        ])
        in_maps.append({
            "xt": xt,
            "w_ih": np.ascontiguousarray(W_ih[rows, :].T).astype(BF16),
            "w_hh": np.ascontiguousarray(W_hh[rows, :].T).astype(BF16),
            "bias": np.ascontiguousarray(bsum[rows].reshape(4, 128)),
            "w_fc": np.ascontiguousarray(
                W_fc[:, p * 128:(p + 1) * 128].T).astype(BF16),
            "ident": eye,
        })
    return in_maps


def postprocess(results, b_fc, T=T_FULL):
    acc = np.zeros((2, T * B), np.float32)
    for r in results:
        acc += r["out"]
    out = acc.reshape(2, T, B).transpose(2, 1, 0) + b_fc[None, None, :]
    return np.ascontiguousarray(out.astype(np.float32))


def kernel(x, W_ih, W_hh, b_ih, b_hh, W_fc, b_fc):
    nc = build_nc(T_FULL)
    in_maps = make_in_maps(x, W_ih, W_hh, b_ih, b_hh, W_fc, T_FULL)
    res = run_bass_kernel_spmd(nc, in_maps, core_ids=list(range(N_CORES)))
    return postprocess(res.results, b_fc, T_FULL)
